# revision 1
# baseline (speedup 1.0000x reference)
"""Causal self-attention (B=2, T=2048, C=1024, H=16, D=64) on 8 trn2 cores.

Sharding: core c handles batch b = c//4 and head group hg = c%4 (heads
4*hg .. 4*hg+3).  Each core computes q/k/v projections for its 4 heads
(as 2 stacked head pairs), causal-softmax attention, and a partial
output projection y_partial = O_heads @ Wo[:, heads].T.  The host sums
the 4 partials per batch and adds the bias.

On-core layout (fp32r matmul operands, fp32 accumulation):
  qT/kT per pair: 4 chunk tiles [128, 512], rows 0:64 head-even,
          64:128 head-odd
  S^T_j = kT[j].T @ qT[I]   (k-major, K=64, both heads of a pair packed
          on PE row tiles 0/64)
  P = exp(S * 1/sqrt(C)) (ACT, batched over pairs of j) * causal mask
  O^T_aug = sum_j Vaug_j.T @ P_j    Vaug = [V_h | ones] -> row 64 of the
          [65, 512] PSUM accumulator is the softmax denominator
  O^T norm: reciprocal (DVE) -> partition_broadcast (GPSIMD) -> mul (DVE)
  y = sum_pairs (O^T stacked).T @ woT  (K=256 over 2 K-tiles of 128)

Emission is interleaved per 512-column chunk (projection chunk c, then
attention for I=c) because the Tile scheduler orders each engine's
instructions by emission priority; y-projection chunks are queued as PE
filler between attention steps (the attention inner loop is ACT-bound).
"""
import numpy as np

import concourse.tile as tile
import concourse.mybir as mybir
from concourse import bacc
from concourse.bass_utils import run_bass_kernel_spmd

FP = mybir.dt.float32
FPR = mybir.dt.float32r
B, T, C = 2, 2048, 1024
H, D = 16, 64
SCALE = 1.0 / 32.0  # 1/sqrt(C)
N_CORES = 8
NKT = C // 128  # 8 K-tiles over the embedding dim
NTK = T // 128  # 16 Tk tiles
NI = T // 512  # 4 Tq chunks
EXP = mybir.ActivationFunctionType.Exp

_nc_cache = {}


def _r(ap):
    """fp32r view of an fp32-layout AP (constants)."""
    return ap.bitcast(FPR)


def build_kernel(repeats=1, hmix=False):
    key = (repeats, hmix)
    if key in _nc_cache:
        return _nc_cache[key]

    nc = bacc.Bacc("TRN2", target_bir_lowering=False, debug=False)

    xT_d = nc.dram_tensor("xT", [C, T], FPR, kind="ExternalInput").ap()
    wqT_d = nc.dram_tensor("wqT", [C, 256], FPR, kind="ExternalInput").ap()
    wkT_d = nc.dram_tensor("wkT", [C, 256], FPR, kind="ExternalInput").ap()
    wvT_d = nc.dram_tensor("wvT", [C, 256], FPR, kind="ExternalInput").ap()
    woT_d = nc.dram_tensor("woT", [256, C], FPR, kind="ExternalInput").ap()
    y_d = nc.dram_tensor("y", [T, C], FP, kind="ExternalOutput").ap()

    # mask_big[p, y] = 1 iff y >= p + 384 : slice [., c0:c0+z+128] with
    # c0 = 384 - z, z = j*128 - I*512 masks diagonal tile j (cols < z are
    # fully below the causal boundary, the next 128 are triangular)
    mask_np = (
        np.arange(896)[None, :] >= (np.arange(128)[:, None] + 384)
    ).astype(np.float32)
    mask_d = nc.inline_tensor(mask_np, "mask_big").ap()
    ident_d = nc.inline_tensor(np.eye(128, dtype=np.float32), "ident").ap()
    ones_d = nc.inline_tensor(np.ones((128, 1), dtype=np.float32), "ones").ap()

    with tile.TileContext(nc) as tc:
        with (
            tc.tile_pool(name="persist", bufs=1) as pp,
            tc.tile_pool(name="xpool", bufs=16) as xpool,
            tc.tile_pool(name="ppool", bufs=4) as ppool,
            tc.tile_pool(name="spool", bufs=4) as spool,
            tc.tile_pool(name="ypool", bufs=4) as ypool,
            tc.tile_pool(name="ps_s", bufs=2, space="PSUM") as ps_s,
            tc.tile_pool(name="ps_o", bufs=2, space="PSUM") as ps_o,
            tc.tile_pool(name="ps_y", bufs=2, space="PSUM") as ps_y,
        ):
            # ---- critical-path DMAs first: wq, then xT chunk 0, then the
            # rest; weight matrices load as ONE rearranged DMA each to keep
            # the serial HWDGE issue path short ----
            wq_big = pp.tile([128, NKT, 256], FPR, tag="wq")
            nc.sync.dma_start(
                wq_big[:, :, :], wqT_d.rearrange("(n p) d -> p n d", p=128)
            )
            wq = [wq_big[:, kk, :] for kk in range(NKT)]
            xts_by_chunk = {0: [None] * NKT, 1: [None] * NKT}
            for kk in range(NKT):
                xt = xpool.tile([128, 512], FPR, tag="xt", name=f"xtc0_{kk}")
                nc.sync.dma_start(xt[:], xT_d[kk * 128 : (kk + 1) * 128, 0:512])
                xts_by_chunk[0][kk] = xt
            wk_big = pp.tile([128, NKT, 256], FPR, tag="wk")
            nc.sync.dma_start(
                wk_big[:, :, :], wkT_d.rearrange("(n p) d -> p n d", p=128)
            )
            wk = [wk_big[:, kk, :] for kk in range(NKT)]
            wv_big = pp.tile([128, NKT, 256], FPR, tag="wv")
            nc.sync.dma_start(
                wv_big[:, :, :], wvT_d.rearrange("(n p) d -> p n d", p=128)
            )
            wv = [wv_big[:, kk, :] for kk in range(NKT)]
            for kk in range(NKT):
                xt = xpool.tile([128, 512], FPR, tag="xt", name=f"xtc1_{kk}")
                nc.sync.dma_start(xt[:], xT_d[kk * 128 : (kk + 1) * 128, 512:1024])
                xts_by_chunk[1][kk] = xt
            wo_big = pp.tile([128, 2, C], FPR, tag="wo")
            nc.sync.dma_start(
                wo_big[:, :, :], woT_d.rearrange("(n p) d -> p n d", p=128)
            )
            wo = [wo_big[:, kk, :] for kk in range(2)]

            mask = pp.tile([128, 896], FP, tag="mask")
            nc.sync.dma_start(mask[:], mask_d[:])
            ones_sb = pp.tile([128, 1], FPR, tag="ones")
            nc.sync.dma_start(ones_sb[:], _r(ones_d[:]))
            ident = pp.tile([128, 128], FP, tag="ident")
            nc.sync.dma_start(ident[:], ident_d[:])

            # ---- persistent activations, chunked per 512 columns ----
            qTc = [
                [pp.tile([128, 512], FPR, tag=f"qT{p}_{i}", name=f"qT{p}_{i}")
                 for i in range(NI)]
                for p in range(2)
            ]
            kTc = [
                [pp.tile([128, 512], FPR, tag=f"kT{p}_{i}", name=f"kT{p}_{i}")
                 for i in range(NI)]
                for p in range(2)
            ]
            vTc = [
                [pp.tile([128, 512], FP, tag=f"vT{p}_{i}", name=f"vT{p}_{i}")
                 for i in range(NI)]
                for p in range(2)
            ]
            otstc = [
                [pp.tile([128, 512], FPR, tag=f"ot{p}_{i}", name=f"otst{p}_{i}")
                 for i in range(NI)]
                for p in range(2)
            ]
            vaug = [
                [
                    pp.tile([128, 130], FPR, tag=f"va{p}_{t}", name=f"vaug{p}_{t}")
                    for t in range(NTK)
                ]
                for p in range(2)
            ]

            # ---- emission helpers ----
            R = [0]
            def emit_xt_chunk(c):
                tiles = []
                for kk in range(NKT):
                    xt = xpool.tile([128, 512], FPR, tag="xt", name=f"xtc{c}_{kk}_r{R[0]}")
                    nc.sync.dma_start(
                        xt[:], xT_d[kk * 128 : (kk + 1) * 128, c * 512 : c * 512 + 512]
                    )
                    tiles.append(xt)
                return tiles

            def emit_proj_chunk(c, xts):
                for wts, dsts, nm in ((wq, qTc, "q"), (wk, kTc, "k"), (wv, vTc, "v")):
                    for pair in range(2):
                        ps = ps_y.tile([128, 512], FP, tag="ps_proj",
                                       name=f"pspr{nm}{c}_{pair}_r{R[0]}")
                        for kk in range(NKT):
                            nc.tensor.matmul(
                                ps[:],
                                lhsT=wts[kk][:, pair * 128 : pair * 128 + 128],
                                rhs=xts[kk][:],
                                start=(kk == 0),
                                stop=(kk == NKT - 1),
                            )
                        nc.vector.tensor_copy(dsts[pair][c][:], ps[:])

            def emit_transposes(c):
                for pair in range(2):
                    for t in range(4 * c, 4 * c + 4):
                        pst = ps_y.tile([128, 128], FP, tag="ps_proj",
                                        name=f"pstr{pair}_{t}_r{R[0]}")
                        nc.tensor.transpose(
                            pst[:],
                            vTc[pair][c][:, (t % 4) * 128 : (t % 4) * 128 + 128],
                            ident[:],
                        )
                        va = vaug[pair][t]
                        nc.vector.tensor_copy(va[:, 64:65], ones_sb[:])
                        nc.vector.tensor_copy(va[:, 129:130], ones_sb[:])
                        if c < 2:  # ACT is idle before attention starts
                            nc.scalar.copy(va[:, 0:64], pst[:, 0:64])
                            nc.scalar.copy(va[:, 65:129], pst[:, 64:128])
                        else:  # mid-attention: ACT is the exp bottleneck
                            nc.vector.tensor_copy(va[:, 0:64], pst[:, 0:64])
                            nc.vector.tensor_copy(va[:, 65:129], pst[:, 64:128])

            fillers = []

            def emit_yproj_chunk(t, nch, on_act=False):
                ps = ps_y.tile([128, 512], FP, tag="ps_proj", name=f"psy{t}_{nch}_r{R[0]}")
                for pair in range(2):
                    nc.tensor.matmul(
                        ps[:],
                        lhsT=otstc[pair][t // 4][
                            :, (t % 4) * 128 : (t % 4) * 128 + 128
                        ],
                        rhs=wo[pair][:, nch * 512 : nch * 512 + 512],
                        start=(pair == 0),
                        stop=(pair == 1),
                    )
                yt = ypool.tile([128, 512], FP, tag="yout", name=f"yt{t}_{nch}_r{R[0]}")
                if on_act:
                    nc.scalar.copy(yt[:], ps[:])
                else:
                    nc.vector.tensor_copy(yt[:], ps[:])
                nc.sync.dma_start(
                    y_d[t * 128 : (t + 1) * 128, nch * 512 : nch * 512 + 512],
                    yt[:],
                )

            nfill = [0]

            def maybe_fill():
                nfill[0] += 1
                if nfill[0] % 4 == 0 and fillers:
                    fillers.pop(0)()

            def emit_attention(I):
                if hmix:
                    emit_attention_hmix(I)
                    return
                jmax = 4 * I + 4
                for pair in range(2):
                    oT = [None, None]
                    for h in (1, 0):
                        oT[h] = ps_o.tile([65, 512], FP, tag="oT",
                                          name=f"o{I}_{pair}_{h}_r{R[0]}")
                        hsl = slice(64 * h, 64 * h + 64)
                        for jb in range(jmax // 2):
                            j0 = 2 * jb
                            diag = j0 >= 4 * I  # both tiles in diagonal region
                            zs = [max(0, (j0 + dj) * 128 - I * 512) for dj in range(2)]
                            s_ps = ps_s.tile([128, 1024], FP, tag="s",
                                             name=f"s{I}_{pair}_{h}_{jb}_r{R[0]}")
                            for dj in range(2):
                                j = j0 + dj
                                z = zs[dj]
                                nc.tensor.matmul(
                                    s_ps[:, dj * 512 + z : dj * 512 + 512],
                                    lhsT=kTc[pair][j // 4][
                                        hsl, (j % 4) * 128 : (j % 4) * 128 + 128
                                    ],
                                    rhs=qTc[pair][I][hsl, z:512],
                                    start=True,
                                    stop=True,
                                )
                            p_sb = ppool.tile([128, 1024], FPR, tag="p",
                                              name=f"p{I}_{pair}_{h}_{jb}_r{R[0]}")
                            if not diag:
                                nc.scalar.activation(p_sb[:], s_ps[:], EXP,
                                                     scale=SCALE)
                            else:
                                # trimmed: columns below the causal boundary
                                # were never computed
                                for dj in range(2):
                                    lo = dj * 512 + zs[dj]
                                    hi = dj * 512 + 512
                                    nc.scalar.activation(
                                        p_sb[:, lo:hi], s_ps[:, lo:hi], EXP,
                                        scale=SCALE,
                                    )
                            for dj in range(2):
                                j = j0 + dj
                                z = zs[dj]
                                if j >= 4 * I:
                                    # triangular strip at the causal boundary
                                    ssl2 = slice(dj * 512 + z, dj * 512 + z + 128)
                                    nc.vector.tensor_mul(
                                        p_sb[:, ssl2], p_sb[:, ssl2],
                                        _r(mask[:, 384:512]),
                                    )
                                nc.tensor.matmul(
                                    oT[h][:, z:512],
                                    lhsT=vaug[pair][j][:, 65 * h : 65 * h + 65],
                                    rhs=p_sb[:, dj * 512 + z : dj * 512 + 512],
                                    start=(j == 0),
                                    stop=(j == jmax - 1),
                                )
                            maybe_fill()
                    # normalize: O^T[0:64] * (1/rowsum) into the stacked chunk
                    for h in (1, 0):
                        recip = spool.tile([1, 512], FP, tag="recip",
                                           name=f"rc{I}_{pair}_{h}_r{R[0]}")
                        nc.vector.reciprocal(recip[:], oT[h][64:65, :])
                        bcast = spool.tile([64, 512], FP, tag="bcast",
                                           name=f"bc{I}_{pair}_{h}_r{R[0]}")
                        nc.gpsimd.partition_broadcast(bcast[:], recip[:])
                        if h == 0:
                            nc.vector.tensor_mul(
                                otstc[pair][I][0:64, :], oT[h][0:64, :], bcast[:]
                            )
                        else:
                            onrm = spool.tile([64, 512], FPR, tag="onrm",
                                              name=f"on{I}_{pair}_r{R[0]}")
                            nc.vector.tensor_mul(onrm[:], oT[h][0:64, :], bcast[:])
                            # partition shift 0->64 needs a DMA
                            nc.sync.dma_start(otstc[pair][I][64:128, :], onrm[:])
                for t in range(4 * I, 4 * I + 4):
                    for nch in range(2):
                        fillers.append(
                            lambda t=t, nch=nch, **kw: emit_yproj_chunk(t, nch, **kw)
                        )

            def emit_attention_hmix(I):
                # Both heads of a pair advance together so that the two K=64
                # S matmuls (PE row groups 0 and 64) are adjacent in the PE
                # stream and can overlap on hardware.
                jmax = 4 * I + 4
                for pair in range(2):
                    oT = []
                    for h in range(2):
                        o = ps_o.tile([65, 512], FP, tag="oT",
                                      name=f"o{I}_{pair}_{h}_r{R[0]}")
                        oT.append(o)
                    for jb in range(jmax // 2):
                        j0 = 2 * jb
                        s_ps = []
                        for h in range(2):
                            sp = ps_s.tile([128, 1024], FP, tag="s",
                                           name=f"s{I}_{pair}_{h}_{jb}_r{R[0]}")
                            s_ps.append(sp)
                        for dj in range(2):
                            j = j0 + dj
                            for h in range(2):
                                hsl = slice(64 * h, 64 * h + 64)
                                nc.tensor.matmul(
                                    s_ps[h][:, dj * 512 : dj * 512 + 512],
                                    lhsT=kTc[pair][j // 4][
                                        hsl, (j % 4) * 128 : (j % 4) * 128 + 128
                                    ],
                                    rhs=qTc[pair][I][hsl, :],
                                    start=True,
                                    stop=True,
                                )
                        p_sb = []
                        for h in range(2):
                            pt = ppool.tile([128, 1024], FPR, tag="p",
                                            name=f"p{I}_{pair}_{h}_{jb}_r{R[0]}")
                            nc.scalar.activation(pt[:], s_ps[h][:], EXP, scale=SCALE)
                            p_sb.append(pt)
                        for dj in range(2):
                            j = j0 + dj
                            for h in range(2):
                                if j >= 4 * I:  # diagonal tile: causal mask
                                    z = j * 128 - I * 512
                                    c0 = 384 - z
                                    msl = slice(dj * 512, dj * 512 + z + 128)
                                    nc.vector.tensor_mul(
                                        p_sb[h][:, msl], p_sb[h][:, msl],
                                        _r(mask[:, c0 : c0 + z + 128]),
                                    )
                                nc.tensor.matmul(
                                    oT[h][:],
                                    lhsT=vaug[pair][j][:, 65 * h : 65 * h + 65],
                                    rhs=p_sb[h][:, dj * 512 : dj * 512 + 512],
                                    start=(j == 0),
                                    stop=(j == jmax - 1),
                                )
                        maybe_fill()
                        maybe_fill()
                    for h in (1, 0):
                        recip = spool.tile([1, 512], FP, tag="recip",
                                           name=f"rc{I}_{pair}_{h}_r{R[0]}")
                        nc.vector.reciprocal(recip[:], oT[h][64:65, :])
                        bcast = spool.tile([64, 512], FP, tag="bcast",
                                           name=f"bc{I}_{pair}_{h}_r{R[0]}")
                        nc.gpsimd.partition_broadcast(bcast[:], recip[:])
                        if h == 0:
                            nc.vector.tensor_mul(
                                otstc[pair][I][0:64, :], oT[h][0:64, :], bcast[:]
                            )
                        else:
                            onrm = spool.tile([64, 512], FPR, tag="onrm",
                                              name=f"on{I}_{pair}_r{R[0]}")
                            nc.vector.tensor_mul(onrm[:], oT[h][0:64, :], bcast[:])
                            nc.sync.dma_start(otstc[pair][I][64:128, :], onrm[:])
                for t in range(4 * I, 4 * I + 4):
                    for nch in range(2):
                        fillers.append(
                            lambda t=t, nch=nch, **kw: emit_yproj_chunk(t, nch, **kw)
                        )

            # ---- interleaved emission: proj chunk c, then attention I=c;
            # attention I=0 (shortest) is slotted after I=2 ----
            for rep in range(repeats):
                R[0] = rep
                for c in range(NI):
                    if rep == 0 and c in xts_by_chunk:
                        xts = xts_by_chunk[c]
                    else:
                        xts = emit_xt_chunk(c)
                    emit_proj_chunk(c, xts)
                    emit_transposes(c)
                    if c >= 1:
                        emit_attention(c)
                emit_attention(0)
                while fillers:
                    fillers.pop(0)(on_act=True)  # tail: ACT is idle here

    nc.compile()
    _nc_cache[key] = nc
    return nc


def make_in_maps(x, Wq, Wk, Wv, Wo):
    x = np.asarray(x, dtype=np.float32)
    Wq = np.asarray(Wq, dtype=np.float32)
    Wk = np.asarray(Wk, dtype=np.float32)
    Wv = np.asarray(Wv, dtype=np.float32)
    Wo = np.asarray(Wo, dtype=np.float32)
    in_maps = []
    for c in range(N_CORES):
        b, hg = c // 4, c % 4
        sl = slice(256 * hg, 256 * hg + 256)
        in_maps.append(
            {
                "xT": np.ascontiguousarray(x[b].T),
                "wqT": np.ascontiguousarray(Wq[sl, :].T),
                "wkT": np.ascontiguousarray(Wk[sl, :].T),
                "wvT": np.ascontiguousarray(Wv[sl, :].T),
                "woT": np.ascontiguousarray(Wo[:, sl].T),
            }
        )
    return in_maps


def run_spmd(in_maps, trace=False, repeats=1, **kw):
    nc = build_kernel(repeats)
    return run_bass_kernel_spmd(nc, in_maps, list(range(N_CORES)), trace=trace, **kw)


def gather(results, bo):
    bo = np.asarray(bo, dtype=np.float32)
    y = np.empty((B, T, C), dtype=np.float32)
    for b in range(B):
        acc = results[4 * b]["y"].astype(np.float32).copy()
        for g in range(1, 4):
            acc += results[4 * b + g]["y"]
        y[b] = acc + bo[None, :]
    return y


def kernel(x, Wq, Wk, Wv, Wo, bo):
    res = run_spmd(make_in_maps(x, Wq, Wk, Wv, Wo))
    return gather(res.results, bo)



# revision 2
# speedup vs baseline: 1.1031x; 1.1031x over previous
"""Causal self-attention (B=2, T=2048, C=1024, H=16, D=64) on 8 trn2 cores.

Sharding: core c handles batch b = c//4 and head group hg = c%4 (heads
4*hg .. 4*hg+3, as 2 pairs).  Each core computes q/k/v projections for
its 4 heads, causal-softmax attention, and a partial output projection
y_partial = O_heads @ Wo[:, heads].T (bf16).  The host sums the 4
partials per batch in fp32 and adds the bias.

Numerics (validated vs the fp32 reference, rel err ~9e-3 < 2e-2):
  q,k projections: fp8e4m3 DoubleRow matmuls (x fp8, W*16 fp8; the 16*16
      scale is absorbed into the exp scale), q/k stored bf16.
  v projection: bf16, computed directly in [T, ch] orientation (lhsT =
      xT tile, rhs = Wv) so no PE transposes are needed; v lands in the
      vaug layout [128 kpos, j, h, 65] with a ones column for the
      softmax denominator.
  S = k.T q (bf16, K=64), P = exp(S * scale) in bf16, causal strip
      masking via one [128,2,128] DVE mul per psum tile (2-byte 2x mode),
  O^T accumulated per head with the denominator in psum row 64,
  normalize via reciprocal (DVE) + partition_broadcast (Pool) + mul,
  y projection bf16; y output bf16 (host sums partials in fp32).
"""
import numpy as np
import ml_dtypes

import concourse.tile as tile
import concourse.mybir as mybir
from concourse import bacc
from concourse.bass_utils import run_bass_kernel_spmd

FP = mybir.dt.float32
BF = mybir.dt.bfloat16
F8 = mybir.dt.float8e4
DR = mybir.MatmulPerfMode.DoubleRow
B, T, C = 2, 2048, 1024
H, D = 16, 64
WS = 16.0  # host-side weight scale for fp8 q/k weights
SCALE = (1.0 / 32.0) / (WS * WS)  # 1/sqrt(C) / (q,k weight scales)
N_CORES = 8
NKT = C // 128  # 8 K-tiles over the embedding dim
NTK = T // 128  # 16 Tk tiles
NI = T // 512  # 4 Tq chunks
EXP = mybir.ActivationFunctionType.Exp

_nc_cache = {}


def build_kernel(repeats=1):
    key = repeats
    if key in _nc_cache:
        return _nc_cache[key]

    nc = bacc.Bacc("TRN2", target_bir_lowering=False, debug=False)

    # host pre-arranges inputs partition-major so every DMA line is >=2KB
    # x8/xlo: [NI*128, NKT*512] chunk-major; weights: [128, NKT*256]
    x8_d = nc.dram_tensor("x8T", [NI * 128, NKT * 512], F8,
                          kind="ExternalInput").ap()
    xlo_d = nc.dram_tensor("xlo8T", [NI * 128, NKT * 512], F8,
                           kind="ExternalInput").ap()
    w8q_d = nc.dram_tensor("w8q", [128, NKT * 256], F8,
                           kind="ExternalInput").ap()
    w8k_d = nc.dram_tensor("w8k", [128, NKT * 256], F8,
                           kind="ExternalInput").ap()
    wvh_d = nc.dram_tensor("wvh8", [128, NKT * 256], F8,
                           kind="ExternalInput").ap()
    wvl_d = nc.dram_tensor("wvl8", [128, NKT * 256], F8,
                           kind="ExternalInput").ap()
    wbo_d = nc.dram_tensor("wbo", [128, 2 * C], BF,
                           kind="ExternalInput").ap()
    y_d = nc.dram_tensor("y", [T, C], BF, kind="ExternalOutput").ap()

    # Triangular strip mask (bf16): pattern[p, y] = 1 iff y >= p, replicated
    # at column 640 so one [128, 2(stride 640), 128] AP covers both strips
    # of a [128, 1024] psum-pair tile.
    tri = (np.arange(128)[None, :] >= np.arange(128)[:, None])
    mask_np = np.zeros((128, 768), dtype=np.float32)
    mask_np[:, 0:128] = tri
    mask_np[:, 640:768] = tri
    mask_d = nc.inline_tensor(
        mask_np.astype(ml_dtypes.bfloat16), "mask_strip"
    ).ap()

    with tile.TileContext(nc) as tc:
        with (
            tc.tile_pool(name="persist", bufs=1) as pp,
            tc.tile_pool(name="ppool", bufs=4) as ppool,
            tc.tile_pool(name="spool", bufs=8) as spool,
            tc.tile_pool(name="ypool", bufs=4) as ypool,
            tc.tile_pool(name="ps_s", bufs=2, space="PSUM") as ps_s,
            tc.tile_pool(name="ps_o", bufs=2, space="PSUM") as ps_o,
            tc.tile_pool(name="ps_y", bufs=2, space="PSUM") as ps_y,
        ):
            # ---- weight + x DMAs; critical-path ones first ----
            x8 = pp.tile([128, NI, NKT, 512], F8, tag="x8")
            xlo = pp.tile([128, NI, NKT, 512], F8, tag="xlo")
            for kk in range(4):
                nc.sync.dma_start(
                    x8[:, 0, 2 * kk: 2 * kk + 2, :].rearrange(
                        "p n d -> p (n d)"),
                    x8_d[0:128, 2 * kk * 512: (2 * kk + 2) * 512],
                )
            w8q = pp.tile([128, NKT, 256], F8, tag="w8q")
            nc.scalar.dma_start(
                w8q[:, :, :].rearrange("p n d -> p (n d)"), w8q_d[:, :]
            )
            w8k = pp.tile([128, NKT, 256], F8, tag="w8k")
            nc.scalar.dma_start(
                w8k[:, :, :].rearrange("p n d -> p (n d)"), w8k_d[:, :]
            )
            wvh = pp.tile([128, NKT, 256], F8, tag="wvh")
            nc.scalar.dma_start(
                wvh[:, :, :].rearrange("p n d -> p (n d)"), wvh_d[:, :]
            )
            wvl = pp.tile([128, NKT, 256], F8, tag="wvl")
            nc.scalar.dma_start(
                wvl[:, :, :].rearrange("p n d -> p (n d)"), wvl_d[:, :]
            )
            nc.sync.dma_start(
                xlo[:, 0, :, :].rearrange("p n d -> p (n d)"),
                xlo_d[0:128, :],
            )
            wbo = pp.tile([128, 2, C], BF, tag="wbo")
            nc.scalar.dma_start(
                wbo[:, :, :].rearrange("p n d -> p (n d)"), wbo_d[:, :]
            )
            for c in range(1, NI):
                nc.sync.dma_start(
                    x8[:, c, :, :].rearrange("p n d -> p (n d)"),
                    x8_d[c * 128: c * 128 + 128, :],
                )
                nc.sync.dma_start(
                    xlo[:, c, :, :].rearrange("p n d -> p (n d)"),
                    xlo_d[c * 128: c * 128 + 128, :],
                )
            mask = pp.tile([128, 768], BF, tag="mask")
            nc.scalar.dma_start(mask[:], mask_d[:])

            # ---- persistent activations ----
            # qT/kT: [d(2 heads packed 0:64/64:128), T] bf16 per pair
            qT = [pp.tile([128, T], BF, tag=f"qT{p}", name=f"qT{p}") for p in range(2)]
            kT = [pp.tile([128, T], BF, tag=f"kT{p}", name=f"kT{p}") for p in range(2)]
            # vaug: [kpos, j, h, 65] bf16 per pair; col 64 of each h-plane
            # is the ones column (softmax denominator via the O matmul)
            vaug = [
                pp.tile([128, NTK, 2, 65], BF, tag=f"va{p}", name=f"vaug{p}") for p in range(2)
            ]
            for p in range(2):
                nc.gpsimd.memset(vaug[p][:, :, :, 64:65], 256.0)
            # otst: [o-ch (h0 0:64 | h1 64:128), T-cols 512] bf16
            otst = [
                [pp.tile([128, 512], BF, tag=f"ot{p}_{i}", name=f"otst{p}_{i}") for i in range(NI)]
                for p in range(2)
            ]

            R = [0]

            def emit_qk_unit(c, wts, dst, nm, pair):
                csl = slice(c * 512, c * 512 + 512)
                ps = ps_y.tile([128, 512], FP, tag="ps_proj",
                               name=f"ps{nm}{c}_{pair}_r{R[0]}")
                for nch in range(2):
                    n0 = nch * 256
                    for kk in range(NKT // 2):
                        nc.tensor.matmul(
                            ps[:, nch * 256: nch * 256 + 256],
                            lhsT=wts[:, 2 * kk: 2 * kk + 2,
                                     pair * 128: pair * 128 + 128],
                            rhs=x8[:, c, 2 * kk: 2 * kk + 2, n0: n0 + 256],
                            start=(kk == 0),
                            stop=(kk == NKT // 2 - 1),
                            perf_mode=DR,
                        )
                if c == 0:
                    nc.scalar.copy(dst[pair][:, csl], ps[:])
                else:
                    nc.vector.tensor_copy(dst[pair][:, csl], ps[:])

            def emit_v_tile(t):
                # v in [T, ch] orientation: per 128-row T-tile, out psum
                # [128 T, 256 ch]; ch = [pair0 h0|h1, pair1 h0|h1]
                tc_, tr = t // 4, (t % 4) * 128
                ps = ps_y.tile([128, 256], FP, tag="ps_proj",
                               name=f"psv{t}_r{R[0]}")
                chains = ((x8, wvh), (xlo, wvh), (x8, wvl))
                for ci, (lhs, w) in enumerate(chains):
                    for kk in range(NKT // 2):
                        nc.tensor.matmul(
                            ps[:],
                            lhsT=lhs[:, tc_, 2 * kk: 2 * kk + 2,
                                     tr: tr + 128],
                            rhs=w[:, 2 * kk: 2 * kk + 2, :],
                            start=(ci == 0 and kk == 0),
                            stop=(ci == 2 and kk == NKT // 2 - 1),
                            perf_mode=DR,
                        )
                for pair in range(2):
                    src = ps[:, pair * 128: pair * 128 + 128]
                    if t < 6:
                        nc.scalar.copy(
                            vaug[pair][:, t, :, 0:64],
                            src.rearrange("p (h d) -> p h d", h=2),
                        )
                    else:
                        nc.vector.tensor_copy(
                            vaug[pair][:, t, :, 0:64],
                            src.rearrange("p (h d) -> p h d", h=2),
                        )

            def qk_units(c):
                return [
                    (lambda c=c, w=w, d=d, nm=nm, p=p:
                     emit_qk_unit(c, w, d, nm, p))
                    for w, d, nm in ((w8q, qT, "q"), (w8k, kT, "k"))
                    for p in range(2)
                ]

            def v_units(c):
                return [(lambda t=t: emit_v_tile(t))
                        for t in range(4 * c, 4 * c + 4)]

            pre_fillers = []  # proj units: prerequisites of a later attention
            y_fillers = []

            yt_tiles = {}

            def emit_yproj_chunk(t, nch, on_act=False):
                ps = ps_y.tile([128, 512], FP, tag="ps_proj",
                               name=f"psy{t}_{nch}_r{R[0]}")
                for pair in range(2):
                    nc.tensor.matmul(
                        ps[:],
                        lhsT=otst[pair][t // 4][
                            :, (t % 4) * 128: (t % 4) * 128 + 128
                        ],
                        rhs=wbo[:, pair, nch * 512: nch * 512 + 512],
                        start=(pair == 0),
                        stop=(pair == 1),
                    )
                if t not in yt_tiles:
                    yt_tiles[t] = ypool.tile([128, 1024], BF, tag="yout",
                                             name=f"yt{t}_r{R[0]}")
                yt = yt_tiles[t]
                if on_act:
                    nc.scalar.copy(yt[:, nch * 512: nch * 512 + 512], ps[:])
                else:
                    nc.vector.tensor_copy(
                        yt[:, nch * 512: nch * 512 + 512], ps[:]
                    )
                if nch == 1:
                    nc.sync.dma_start(
                        y_d[t * 128: (t + 1) * 128, :], yt[:, :]
                    )
                    del yt_tiles[t]

            nfill = [0]
            pop_every = [1]
            ycount = [0]

            allow_y = [False]

            def maybe_fill():
                nfill[0] += 1
                if nfill[0] % pop_every[0] == 0:
                    if pre_fillers:
                        pre_fillers.pop(0)[1]()
                    elif y_fillers and allow_y[0]:
                        y_fillers.pop(0)()

            def emit_attention(I):
                jmax = 4 * I + 4
                for pair in range(2):
                    oT = [None, None]
                    for h in (1, 0):
                        oT[h] = ps_o.tile([65, 512], FP, tag="oT",
                                          name=f"o{I}_{pair}_{h}_r{R[0]}")

                    def emit_o(jb, p_sb, zs):
                        for h in (1, 0):
                            for dj in range(2):
                                j = 2 * jb + dj
                                z = zs[dj]
                                nc.tensor.matmul(
                                    oT[h][:, z:512],
                                    lhsT=vaug[pair][:, j, h, :],
                                    rhs=p_sb[h][:, dj * 512 + z:
                                                dj * 512 + 512],
                                    start=(j == 0),
                                    stop=(j == jmax - 1),
                                )

                    prev = None
                    for jb in range(jmax // 2):
                        j0 = 2 * jb
                        diag = j0 >= 4 * I
                        zs = [max(0, (j0 + dj) * 128 - I * 512)
                              for dj in range(2)]
                        s_ps = {}
                        for h in (1, 0):
                            s_ps[h] = ps_s.tile(
                                [128, 1024], FP, tag="s",
                                name=f"s{I}_{pair}_{h}_{jb}_r{R[0]}")
                        # both heads' S adjacent per dj: K=64 row-tiles
                        # (partitions 0:64 / 64:128) run concurrently on HW
                        for dj in range(2):
                            j = j0 + dj
                            z = zs[dj]
                            for h in (1, 0):
                                hsl = slice(64 * h, 64 * h + 64)
                                nc.tensor.matmul(
                                    s_ps[h][:, dj * 512 + z: dj * 512 + 512],
                                    lhsT=kT[pair][hsl,
                                                  j * 128: j * 128 + 128],
                                    rhs=qT[pair][hsl, I * 512 + z:
                                                 I * 512 + 512],
                                    start=True,
                                    stop=True,
                                )
                        p_sb = {}
                        for h in (1, 0):
                            p_sb[h] = ppool.tile(
                                [128, 1024], BF, tag="p",
                                name=f"p{I}_{pair}_{h}_{jb}_r{R[0]}")
                            if not diag:
                                nc.scalar.activation(p_sb[h][:], s_ps[h][:],
                                                     EXP, scale=SCALE)
                            elif zs[0] == 0:
                                # [0:512] and [640:1024] merged; the gap
                                # holds stale bounded S values, its exp is
                                # finite and never read by the O matmuls
                                nc.scalar.activation(p_sb[h][:], s_ps[h][:],
                                                     EXP, scale=SCALE)
                            else:
                                for dj in range(2):
                                    lo = dj * 512 + zs[dj]
                                    hi = dj * 512 + 512
                                    nc.scalar.activation(
                                        p_sb[h][:, lo:hi], s_ps[h][:, lo:hi],
                                        EXP, scale=SCALE,
                                    )
                            if diag:
                                for dj in range(2):
                                    ssl = slice(dj * 512 + zs[dj],
                                                dj * 512 + zs[dj] + 128)
                                    nc.vector.tensor_mul(
                                        p_sb[h][:, ssl], p_sb[h][:, ssl],
                                        mask[:, 0:128],
                                    )
                        maybe_fill()
                        if prev is not None:
                            emit_o(*prev)
                        prev = (jb, p_sb, zs)
                    emit_o(*prev)
                    for h in (1, 0):
                        recip = spool.tile([1, 512], FP, tag="recip",
                                           name=f"rc{I}_{pair}_{h}_r{R[0]}")
                        nc.vector.reciprocal(recip[:], oT[h][64:65, :])
                        bcast = spool.tile([64, 512], FP, tag="bcast",
                                           name=f"bc{I}_{pair}_{h}_r{R[0]}")
                        nc.gpsimd.partition_broadcast(bcast[:], recip[:])
                        if h == 0:
                            nc.vector.tensor_mul(
                                otst[pair][I][0:64, :], oT[h][0:64, :],
                                bcast[:],
                            )
                        else:
                            onrm = spool.tile([64, 512], BF, tag="onrm",
                                              name=f"on{I}_{pair}_r{R[0]}")
                            nc.vector.tensor_mul(onrm[:], oT[h][0:64, :],
                                                 bcast[:])
                            # partition shift 0->64 needs a DMA
                            nc.sync.dma_start(otst[pair][I][64:128, :],
                                              onrm[:])
                for t in range(4 * I, 4 * I + 4):
                    for nch in range(2):
                        y_fillers.append(
                            lambda t=t, nch=nch, **kw:
                            emit_yproj_chunk(t, nch, **kw)
                        )

            # ---- interleaved emission: attention I consumes chunk-I+1
            # projection units (and earlier y units) as PE fillers inside
            # the exp-latency window of each jb step ----
            for rep in range(repeats):
                R[0] = rep
                for u in qk_units(0):
                    u()
                v0 = v_units(0)
                v0[0]()
                v0[1]()
                pre_fillers.extend((0, u) for u in v0[2:])
                for c in range(1, NI):
                    pre_fillers.extend((c, u) for u in qk_units(c))
                    pre_fillers.extend((c, u) for u in v_units(c))
                for I in range(NI):
                    # y units are deferred to the last attention, whose own
                    # filler supply is otherwise thin
                    allow_y[0] = I == NI - 1
                    slots = 4 * (2 * I + 2)
                    units = len(pre_fillers) + (
                        len(y_fillers) if allow_y[0] else 0
                    )
                    pop_every[0] = max(1, slots // max(1, units))
                    nfill[0] = 0
                    emit_attention(I)
                    # chunk I+1 units must complete before attention I+1
                    while pre_fillers and pre_fillers[0][0] <= I + 1:
                        pre_fillers.pop(0)[1]()
                na = [0]
                while y_fillers:
                    na[0] += 1
                    y_fillers.pop(0)(on_act=(na[0] % 2 == 0))

    nc.compile()
    _nc_cache[key] = nc
    return nc


def make_in_maps(x, Wq, Wk, Wv, Wo):
    x = np.asarray(x, dtype=np.float32)
    Wq = np.asarray(Wq, dtype=np.float32)
    Wk = np.asarray(Wk, dtype=np.float32)
    Wv = np.asarray(Wv, dtype=np.float32)
    Wo = np.asarray(Wo, dtype=np.float32)
    f8 = ml_dtypes.float8_e4m3fn
    bf = ml_dtypes.bfloat16
    in_maps = []
    for c in range(N_CORES):
        b, hg = c // 4, c % 4
        sl = slice(256 * hg, 256 * hg + 256)
        xT = np.ascontiguousarray(x[b].T)
        x8 = xT.astype(f8)
        xlo = (xT - x8.astype(np.float32)).astype(f8)

        def xarr(a):  # [C, T] -> [NI*128, NKT*512] chunk-major
            return np.ascontiguousarray(
                a.reshape(8, 128, 4, 512).transpose(2, 1, 0, 3)
            ).reshape(4 * 128, 8 * 512)

        def warr(a):  # [C, 256] -> [128, NKT*256]
            return np.ascontiguousarray(
                a.reshape(8, 128, 256).transpose(1, 0, 2)
            ).reshape(128, 8 * 256)

        wvh = (256.0 * Wv[sl, :].T).astype(f8)
        wvl = ((256.0 * Wv[sl, :].T) - wvh.astype(np.float32)).astype(f8)
        wboT = Wo[:, sl].T.astype(bf)  # [256, C]
        in_maps.append(
            {
                "x8T": xarr(x8),
                "xlo8T": xarr(xlo),
                "w8q": warr((WS * Wq[sl, :]).T.astype(f8)),
                "w8k": warr((WS * Wk[sl, :]).T.astype(f8)),
                "wvh8": warr(wvh),
                "wvl8": warr(wvl),
                "wbo": np.ascontiguousarray(
                    wboT.reshape(2, 128, C).transpose(1, 0, 2)
                ).reshape(128, 2 * C),
            }
        )
    return in_maps


def run_spmd(in_maps, trace=False, repeats=1, **kw):
    nc = build_kernel(repeats)
    return run_bass_kernel_spmd(nc, in_maps, list(range(N_CORES)),
                                trace=trace, **kw)


def gather(results, bo):
    bo = np.asarray(bo, dtype=np.float32)
    y = np.empty((B, T, C), dtype=np.float32)
    for b in range(B):
        acc = results[4 * b]["y"].astype(np.float32)
        for g in range(1, 4):
            acc = acc + results[4 * b + g]["y"].astype(np.float32)
        y[b] = acc + bo[None, :]
    return y


def kernel(x, Wq, Wk, Wv, Wo, bo):
    res = run_spmd(make_in_maps(x, Wq, Wk, Wv, Wo))
    return gather(res.results, bo)
